# revision 55
# baseline (speedup 1.0000x reference)
"""Cross-attention kernel for Trainium2, 8 NeuronCores, data-parallel over
batch (B=8 == n_cores, one batch element per core, no collectives).

v3 design (evolved 584us f32r -> 266us v1 -> ~194us measured; traced-mode
numbers in comments below are ~1.2x the untraced harness timing):
  - S^T computed per head-PAIR with two CONCURRENT row-tiled matmuls
    (tile_position (0,0) / (64,0), 64x128 mode): even head -> cols 0:512,
    odd head -> cols 512:1024 of ONE [128,1024] psum tile; ONE exp per
    pair-chunk tile.  Row-tile concurrency measured at 3ns issue spacing
    on this HW -- the S phase runs ~2x the serial rate.
  - Uniform i-tail: KT padded to 640 cols (zeros), V3 tail rows 64:128
    zeroed (incl. ones col) so the padded i rows contribute exp(0)*0 = 0
    to both y and Z.  No special tail mode, no V3 row duplication.
  - Softmax denominator via ones-augmented V (psum row 64 of AV = Z).
    Z rows staged straight off psum (Act engine for most pairs -- fast
    psum read, spare capacity), ONE shared reciprocal_approx_fast +
    ONE bf16 cast per pair on DVE, rank-1 PE broadcast as two concurrent
    (32,64)-tile matmuls per t-half, ONE pair-wide DVE multiply of YT.
  - Pair 7's broadcast matmuls are woven between out-proj tt=0's cc=6/7
    accumulation matmuls so its DVE chain hides under real PE work.
  - bo folded into the out-proj drain (DVE tensor_add with a broadcast
    bias tile); no rank-1 bias matmuls, no (32,128)-mode flips.
  - Out-proj psum rotates in the (idle by then) 2-buf S pool so tt+1's
    matmuls overlap tt's DVE drain.  Projection psum is TWO 1-bank
    [128,512] tiles (same 2-bank footprint) so the k->q->v allocation
    chain ping-pongs instead of serializing on each DVE drain.
  - Inputs loaded with FEW large DMAs (HWDGE issue costs ~1.2us each),
    need-ordered across the sync + scalar(Act) rings; wo reuses wv's SBUF
    via tag sharing (its DMA self-gates on the last V-proj matmul).  Tiny
    bias gathers ride the gpsimd SWDGE ring after its memsets.
  - Short warmup (HAM un-throttle) that gates only on two small memsets.
"""

import numpy as np

T = 1024
C = 1024
I = 576
IP = 640                # padded i (5 full 128-chunks)
H = 16
D = 64
NCC = C // 128          # 8 contraction chunks
VW = 66                 # per-head column block in V tile: 64 v cols + ones col + pad
SCALE = 1.0 / np.sqrt(D)

_CACHE = {}


def _build_nc():
    import concourse.bass as bass
    import concourse.bacc as bacc
    import concourse.mybir as mybir
    import concourse.tile as tile
    from contextlib import ExitStack

    f32 = mybir.dt.float32
    bf16 = mybir.dt.bfloat16

    nc = bacc.Bacc()

    xT_d = nc.dram_tensor("xT", [C, T], bf16, kind="ExternalInput")
    encT_d = nc.dram_tensor("encT", [C, I], bf16, kind="ExternalInput")
    wqT_d = nc.dram_tensor("wqT", [C, C], bf16, kind="ExternalInput")
    wkT_d = nc.dram_tensor("wkT", [C, C], bf16, kind="ExternalInput")
    wvT_d = nc.dram_tensor("wvT", [C, C], bf16, kind="ExternalInput")
    woT_d = nc.dram_tensor("woT", [C, C], bf16, kind="ExternalInput")
    bq_d = nc.dram_tensor("bq", [C], f32, kind="ExternalInput")
    bk_d = nc.dram_tensor("bk", [C], f32, kind="ExternalInput")
    bvb_d = nc.dram_tensor("bvb", [128, C], f32, kind="ExternalInput")
    bobb_d = nc.dram_tensor("bobb", [128, C], bf16, kind="ExternalInput")
    out_d = nc.dram_tensor("out", [T, C], bf16, kind="ExternalOutput")

    with ExitStack() as ctx:
        tc = ctx.enter_context(tile.TileContext(nc))

        resid = ctx.enter_context(tc.tile_pool(name="resid", bufs=1))
        misc = ctx.enter_context(tc.tile_pool(name="misc", bufs=1))
        ps_s = ctx.enter_context(tc.tile_pool(name="ps_s", bufs=2, space="PSUM"))
        ps_av = ctx.enter_context(tc.tile_pool(name="ps_av", bufs=1, space="PSUM"))
        ps_pj = ctx.enter_context(tc.tile_pool(name="ps_pj", bufs=2, space="PSUM"))
        exps = ctx.enter_context(tc.tile_pool(name="exps", bufs=27))
        osb = ctx.enter_context(tc.tile_pool(name="osb", bufs=2))
        wvp = ctx.enter_context(tc.tile_pool(name="wvp", bufs=1))
        zrp = ctx.enter_context(tc.tile_pool(name="zrp", bufs=2))

        # ---- resident tensors ----
        zmask = misc.tile([33, 64], bf16)     # rows 0 / 32 used as rank-1 ones
        warm = misc.tile([1, 512], bf16)
        bq_t = misc.tile([128, NCC], f32)
        bk_t = misc.tile([128, NCC], f32)
        bvb = misc.tile([128, C], f32)
        bobc = misc.tile([128, C], bf16)

        wk = resid.tile([128, NCC, C], bf16, name="wk")
        wq = resid.tile([128, NCC, C], bf16, name="wq")
        xT = resid.tile([128, NCC, T], bf16, name="xT")
        encT = resid.tile([128, NCC, IP], bf16, name="encT")
        QT = [resid.tile([128, T], bf16, tag=f"QT{i}", name=f"QT{i}") for i in range(NCC)]
        KT = [resid.tile([128, IP], bf16, tag=f"KT{i}", name=f"KT{i}") for i in range(NCC)]
        V3 = [resid.tile([128, H, VW], bf16, tag=f"V{i}", name=f"V{i}") for i in range(5)]
        YT = [resid.tile([128, T], bf16, tag=f"YT{i}", name=f"YT{i}") for i in range(NCC)]
        # wo reuses wv's SBUF (same tag, bufs=1): its DMA waits until the
        # last V-proj matmul has consumed wv.
        wv = wvp.tile([128, NCC, C], bf16, tag="wv", name="wv")

        # ---- gpsimd: memsets first (two small ones gate the PE warmup),
        # then the tiny bias gathers on the SWDGE ring ----
        nc.gpsimd.memset(zmask, 1.0)
        nc.gpsimd.memset(warm, 1.0)
        nc.gpsimd.dma_start(out=bq_t, in_=bq_d[:].rearrange("(oc p) -> p oc", p=128))
        nc.gpsimd.dma_start(out=bk_t, in_=bk_d[:].rearrange("(oc p) -> p oc", p=128))
        for oc in range(NCC):
            nc.gpsimd.memset(KT[oc][:, I:IP], 0.0)
        for ii in range(4):
            nc.gpsimd.memset(V3[ii][:, :, 64:65], 1.0)
        nc.gpsimd.memset(V3[4], 0.0)
        nc.gpsimd.memset(V3[4][0:64, :, 64:65], 1.0)

        # ---- input DMAs: few big transfers, need-ordered ----
        wkT_r = wkT_d[:, :].rearrange("(cc p) c -> p cc c", p=128)
        wqT_r = wqT_d[:, :].rearrange("(cc p) c -> p cc c", p=128)
        wvT_r = wvT_d[:, :].rearrange("(cc p) c -> p cc c", p=128)
        woT_r = woT_d[:, :].rearrange("(cc p) c -> p cc c", p=128)
        xT_r = xT_d[:, :].rearrange("(cc p) t -> p cc t", p=128)
        encT_r = encT_d[:, :].rearrange("(cc p) i -> p cc i", p=128)

        # k_proj(0/1) gate on wk cols 0:256 + encT; q_half(0,·) on wq cols
        # 0:128 + the xT halves (one per ring); wv by the pre-loop v-groups
        # (~28us); wk cols 256: only by k_proj(2) in round 0 (~45us).
        nc.sync.dma_start(out=wk[:, :, 0:128], in_=wkT_r[:, :, 0:128])
        nc.sync.dma_start(out=wk[:, :, 128:256], in_=wkT_r[:, :, 128:256])
        nc.sync.dma_start(out=xT[:, :, 0:512], in_=xT_r[:, :, 0:512])
        nc.sync.dma_start(out=bvb, in_=bvb_d[:, :])
        nc.sync.dma_start(out=wv, in_=wvT_r)
        nc.sync.dma_start(out=wk[:, :, 256:C], in_=wkT_r[:, :, 256:C])
        nc.sync.dma_start(out=bobc, in_=bobb_d[:, :])

        nc.scalar.dma_start(out=encT[:, :, 0:I], in_=encT_r)
        nc.scalar.dma_start(out=wq[:, :, 0:128], in_=wqT_r[:, :, 0:128])
        nc.scalar.dma_start(out=xT[:, :, 512:T], in_=xT_r[:, :, 512:T])
        nc.scalar.dma_start(out=wq[:, :, 128:256], in_=wqT_r[:, :, 128:256])
        nc.scalar.dma_start(out=wq[:, :, 256:C], in_=wqT_r[:, :, 256:C])

        # ---- PE p-state warmup while the first input DMAs stream ----
        for w in range(10):
            pw = ps_pj.tile([128, 512], f32, tag="pj")
            nc.tensor.matmul(pw[:64, 0:512], zmask[0:1, 0:64], warm,
                             start=True, stop=True)

        # ---- building blocks ----
        def k_proj(oc):
            pk = [ps_pj.tile([128, 512], f32, tag="pj", name=f"pk{ih}")
                  for ih in range(2)]
            for cc in range(NCC):
                for ih in range(2):
                    nc.tensor.matmul(
                        pk[ih][:, 0:288],
                        wk[:, cc, oc * 128 : (oc + 1) * 128],
                        encT[:, cc, ih * 288 : (ih + 1) * 288],
                        start=(cc == 0),
                        stop=(cc == NCC - 1),
                    )
            for ih in range(2):
                nc.vector.tensor_scalar_add(
                    KT[oc][:, ih * 288 : (ih + 1) * 288],
                    pk[ih][:, 0:288],
                    bk_t[:, oc : oc + 1],
                )

        def q_half(oc, tch):
            tsl = slice(tch * 512, (tch + 1) * 512)
            pq = ps_pj.tile([128, 512], f32, tag="pj")
            for cc in range(NCC):
                nc.tensor.matmul(
                    pq[:, 0:512],
                    wq[:, cc, oc * 128 : (oc + 1) * 128],
                    xT[:, cc, tsl],
                    start=(cc == 0),
                    stop=(cc == NCC - 1),
                )
            nc.vector.tensor_scalar_add(QT[oc][:, tsl], pq[:, 0:512], bq_t[:, oc : oc + 1])

        def v_group(och, ii):
            """V3[ii][:, 8*och:8*och+8, 0:64] = (enc @ WvT + bv) block."""
            pi = 128 if ii < 4 else 64
            osl = slice(och * 512, (och + 1) * 512)
            pv = ps_pj.tile([128, 512], f32, tag="pj")
            for cc in range(NCC):
                nc.tensor.matmul(
                    pv[:pi, 0:512],
                    encT[:, cc, ii * 128 : ii * 128 + pi],
                    wv[:, cc, osl],
                    start=(cc == 0),
                    stop=(cc == NCC - 1),
                )
            nc.vector.tensor_add(
                V3[ii][:pi, och * 8 : och * 8 + 8, 0:64],
                pv[:pi, 0:512].rearrange("p (h d) -> p h d", d=64),
                bvb[:pi, osl].rearrange("p (h d) -> p h d", d=64),
            )

        def s_slot(oc, ii, tch):
            """Both heads of pair oc, one i-chunk, one t-half: 2 concurrent
            row-tiled matmuls into one psum tile + ONE exp."""
            sp = ps_s.tile([128, 1024], f32, tag="s")
            for hb, cs in ((0, 0), (64, 512)):
                nc.tensor.matmul(
                    sp[:, cs : cs + 512],
                    KT[oc][hb : hb + 64, ii * 128 : (ii + 1) * 128],
                    QT[oc][hb : hb + 64, tch * 512 : (tch + 1) * 512],
                    start=True,
                    stop=True,
                )
            e = exps.tile([128, 1024], bf16, tag="exps")
            nc.scalar.activation(
                e, sp, mybir.ActivationFunctionType.Exp, scale=float(SCALE)
            )
            return e

        def av_half(h, E, tch, py):
            cs = (h % 2) * 512
            tsl = slice(tch * 512, (tch + 1) * 512)
            for ii in range(5):
                nc.tensor.matmul(
                    py[:65, tsl],
                    V3[ii][:, h, 0:65],
                    E[ii][tch][:, cs : cs + 512],
                    start=(ii == 0),
                    stop=(ii == 4),
                )

        zts = {}
        zextra = {}

        def av_drain(h, py, last=False):
            """Z row staged by the Act engine (fast psum read, idle capacity);
            y rows to YT on DVE (Act for the final pair, freeing DVE for the
            closing 1/Z chain)."""
            oc, hb = h // 2, (h % 2) * 64
            zb = (h % 2) * 32
            if oc not in zrr:
                zrr[oc] = zrp.tile([33, T], bf16, tag="zr", name=f"zr{oc}")
                zts[oc] = zrp.tile([33, T], f32, tag="zt", name=f"zt{oc}")
            if h % 2 == 0 and oc < 6:
                nc.scalar.copy(zts[oc][zb : zb + 1, :], py[64:65, :])
            else:
                nc.vector.tensor_copy(zts[oc][zb : zb + 1, :], py[64:65, :])
            if last:
                nc.scalar.copy(YT[oc][hb : hb + 64, :], py[:64, :])
            else:
                nc.vector.tensor_copy(YT[oc][hb : hb + 64, :], py[:64, :])

        zrr = {}

        def norm_mm(oc):
            """pb[0:64] = 1/Z_even bcast, pb[64:128] = 1/Z_odd bcast via two
            concurrent (32,64)-tile rank-1 matmuls per t-half, then ONE
            pair-wide DVE multiply of YT[oc]."""
            nc.vector.reciprocal_approx_fast(out=zts[oc], in_=zts[oc])
            nc.vector.tensor_copy(zrr[oc], zts[oc])
            zr2 = zrr[oc]
            pb = ps_av.tile([128, 1024], f32, tag="av")
            for tch in range(2):
                tsl = slice(tch * 512, (tch + 1) * 512)
                nc.tensor.matmul(
                    pb[0:64, tsl], zmask[0:1, :], zr2[0:1, tsl],
                    start=True, stop=True,
                )
                nc.tensor.matmul(
                    pb[64:128, tsl], zmask[32:33, :], zr2[32:33, tsl],
                    start=True, stop=True,
                )
            nc.vector.tensor_mul(YT[oc], YT[oc], pb)

        # ---- pre-loop: projections for pairs 0/1, S for pair 0, V och=0 ----
        E = {p: [[None, None] for _ in range(5)] for p in range(8)}

        def s_emit(p, ii, tch):
            E[p][ii][tch] = s_slot(p, ii, tch)

        k_proj(0)
        k_proj(1)
        q_half(0, 0)
        for ii in range(5):
            s_emit(0, ii, 0)
        q_half(0, 1)
        for ii in range(5):
            s_emit(0, ii, 1)
            v_group(0, ii)
        q_half(1, 0)
        q_half(1, 1)
        for ii in range(3):
            v_group(1, ii)

        # wo loads into wv's recycled SBUF (gates itself on the last
        # V-proj matmul); needed from round 6 for the early out-proj start.
        wo = wvp.tile([128, NCC, C], bf16, tag="wv", name="wo")
        nc.sync.dma_start(out=wo, in_=woT_r)
        po0 = [None, None]
        po1 = [None, None]

        def opj_tt0(ccs):
            for cc in ccs:
                for och in range(2):
                    nc.tensor.matmul(
                        po0[och][:, 0:512],
                        YT[cc][:, 0:128],
                        wo[:, cc, och * 512 : (och + 1) * 512],
                        start=(cc == 0),
                        stop=(cc == NCC - 1),
                    )

        def opj_rows(tt, ccs, po):
            for cc in ccs:
                for och in range(2):
                    osl = slice(och * 512, (och + 1) * 512)
                    nc.tensor.matmul(
                        po[:, osl],
                        YT[cc][:, tt * 128 : (tt + 1) * 128],
                        wo[:, cc, osl],
                        start=(cc == 0),
                        stop=(cc == NCC - 1),
                    )

        # ---- main rounds ----
        # Round oc: av(pair oc); S(pair oc+1) t-half 1 + S(pair oc+2) t-half
        # 0 (its K/Q proj drains mid-round); projections for pair oc+2.
        # The half-early S emission keeps the Act exp queue fed so the last
        # pairs' AVs aren't exp-gated once projection work runs out.
        for oc in range(8):
            p1, p2 = oc + 1, oc + 2
            live1, live2 = p1 < 8, p2 < 8
            last = oc == 7
            if live1:
                s_emit(p1, 0, 0)
                s_emit(p1, 1, 0)
            if live2:
                k_proj(p2)
            if live1:
                s_emit(p1, 2, 0)
            py0 = ps_av.tile([128, 1024], f32, tag="av")
            av_half(2 * oc, E[oc], 0, py0)
            if live1:
                s_emit(p1, 3, 0)
            if oc == 0:
                v_group(1, 3)
            av_half(2 * oc, E[oc], 1, py0)
            av_drain(2 * oc, py0, last=last)
            if last:
                po1[0] = ps_s.tile([128, 1024], f32, tag="s", name="po1")
                opj_rows(1, range(6), po1[0])
            if live1:
                s_emit(p1, 4, 0)
                s_emit(p1, 0, 1)
            if live2:
                q_half(p2, 0)
                q_half(p2, 1)
            if oc == 0:
                v_group(1, 4)
            if live1:
                s_emit(p1, 1, 1)
            py1 = ps_av.tile([128, 1024], f32, tag="av")
            av_half(2 * oc + 1, E[oc], 0, py1)
            if live1:
                s_emit(p1, 2, 1)
                s_emit(p1, 3, 1)
            av_half(2 * oc + 1, E[oc], 1, py1)
            av_drain(2 * oc + 1, py1, last=last)
            if last:
                po1[1] = ps_s.tile([128, 1024], f32, tag="s", name="po2")
                opj_rows(2, range(6), po1[1])
            if live1:
                s_emit(p1, 4, 1)
            if not last:
                norm_mm(oc)
            if oc == 6:
                # rounds 6/7 have no projection work left and the PE runs
                # dry on exp-gated S/AV; fill with out-proj tt=0 cc 0..5
                # (pairs 0..5 are already normalized) in the idle proj pool.
                po0[0] = ps_pj.tile([128, 512], f32, tag="pj", name="po0a")
                po0[1] = ps_pj.tile([128, 512], f32, tag="pj", name="po0b")
                opj_tt0(range(6))

        # ---- output projection ----
        # tt=0 was started in round 6 (cc 0..5); finish cc=6, weave pair 7's
        # norm matmuls so its DVE recip/mul chain hides under cc work, then
        # cc=7 and the drain.  tt 1..7 rotate in the (now idle) 2-buf S psum
        # pool so tt+1's matmuls overlap tt's DVE drain.
        opj_tt0([6])
        norm_mm(7)
        opj_tt0([7])
        ot0 = osb.tile([128, C], bf16, tag="osb")
        nc.vector.tensor_add(ot0[:, 0:512], po0[0][:, 0:512], bobc[:, 0:512])
        nc.vector.tensor_add(ot0[:, 512:C], po0[1][:, 0:512], bobc[:, 512:C])
        nc.sync.dma_start(out=out_d[0:128], in_=ot0)
        opj_rows(1, [6, 7], po1[0])
        ot1 = osb.tile([128, C], bf16, tag="osb")
        nc.vector.tensor_add(ot1, po1[0], bobc)
        nc.scalar.dma_start(out=out_d[128:256], in_=ot1)
        opj_rows(2, [6, 7], po1[1])
        ot2 = osb.tile([128, C], bf16, tag="osb")
        nc.vector.tensor_add(ot2, po1[1], bobc)
        nc.sync.dma_start(out=out_d[256:384], in_=ot2)
        for tt in range(3, 8):
            po = ps_s.tile([128, 1024], f32, tag="s")
            for cc in range(NCC):
                for och in range(2):
                    osl = slice(och * 512, (och + 1) * 512)
                    nc.tensor.matmul(
                        po[:, osl],
                        YT[cc][:, tt * 128 : (tt + 1) * 128],
                        wo[:, cc, osl],
                        start=(cc == 0),
                        stop=(cc == NCC - 1),
                    )
            ot = osb.tile([128, C], bf16, tag="osb")
            nc.vector.tensor_add(ot, po, bobc)
            eng = nc.sync if tt % 2 == 0 else nc.scalar
            eng.dma_start(out=out_d[tt * 128 : (tt + 1) * 128], in_=ot)

    nc.compile()
    return nc


def _get_nc():
    if "nc" not in _CACHE:
        _CACHE["nc"] = _build_nc()
    return _CACHE["nc"]


def _prep_in_maps(x, encoder_output, Wq, bq, Wkv, bkv, Wo, bo):
    import ml_dtypes
    f = np.float32
    bf = ml_dtypes.bfloat16
    x = np.asarray(x, f)
    enc = np.asarray(encoder_output, f)
    wqT = np.ascontiguousarray(np.asarray(Wq, f).T.astype(bf))
    wkv = np.asarray(Wkv, f)
    wkT = np.ascontiguousarray(wkv[:C].T.astype(bf))
    wvT = np.ascontiguousarray(wkv[C:].T.astype(bf))
    woT = np.ascontiguousarray(np.asarray(Wo, f).T.astype(bf))
    bq = np.asarray(bq, f)
    bkv = np.asarray(bkv, f)
    bo = np.asarray(bo, f)
    bvb = np.ascontiguousarray(np.broadcast_to(bkv[C:], (128, C)).astype(f))
    bobb = np.ascontiguousarray(np.broadcast_to(bo, (128, C)).astype(bf))
    shared = {
        "wqT": wqT, "wkT": wkT, "wvT": wvT, "woT": woT,
        "bq": bq, "bk": np.ascontiguousarray(bkv[:C]),
        "bvb": bvb, "bobb": bobb,
    }
    return [
        dict(
            shared,
            xT=np.ascontiguousarray(x[b].T.astype(bf)),
            encT=np.ascontiguousarray(enc[b].T.astype(bf)),
        )
        for b in range(x.shape[0])
    ]


def kernel(x, encoder_output, Wq, bq, Wkv, bkv, Wo, bo):
    from concourse.bass_utils import run_bass_kernel_spmd

    nc = _get_nc()
    in_maps = _prep_in_maps(x, encoder_output, Wq, bq, Wkv, bkv, Wo, bo)
    res = run_bass_kernel_spmd(nc, in_maps, list(range(len(in_maps)))).results
    return np.stack([res[b]["out"] for b in range(len(res))]).astype(np.float32)
